# revision 68
# baseline (speedup 1.0000x reference)
"""Trainium2 Bass kernel for nn_AttentionHead (B=4, n_ctx=4096, d_model=1024,
d_hidden=64, causal, scale=1/sqrt(d_model)).

Sharding: 8 cores = 4 batches x 2 balanced causal shards. Core (b, s) handles
the 2048 query rows in 64-row chunks with chunk%2 == s. Keys/x-columns are
permuted per core (my-parity chunks first within each 512-key ntile) so that
every core runs the IDENTICAL SPMD program:

  - slot j (0..3) = 512 queries = my chunks of ntiles 2j, 2j+1
  - slot j attends k-tiles t = 0..8(j+1)-1 (128 permuted keys each)
  - k-tiles t < 8j are fully open; t = 8j + r (r in 0..7) get an additive
    causal mask that depends only on (r, s) -> 8 mask tiles per core, sent
    as data.

Per-core pipeline:
  A: KT/VT = [Wk;Wov] @ xT (bf16, weights stationary, PSUM-accumulated over
     8 d_model chunks); bias-add -> vt_sb bf16; K rows recast to fp8 (Pool)
     into k8_sb; Q likewise from each ntile's first 256 columns -> q8_sb fp8
     (with a zeroed second half for the DoubleRow trick). V transposed to
     natural [k,64] bf16 layout via PE transpose, with an appended ones
     column so attn@[V,1] also yields the softmax denominator.
  B: S^T[k,q] = K8_tile^T @ Q8_slot in ONE fp8 DoubleRow matmul per k-tile
     (second contraction tile = next K tile x zeros => 0.5 cycles/row, 2x
     over bf16); additive -960 mask (fp8 -240 tiles against a 4*I fp8
     identity, DoubleRow again) for diagonal tiles; exp((S+M)/32) on ACT
     over the pair -> SBUF bf16 (no row-max subtraction needed:
     |scores/32| <~ 1.5, and exp(-960/32) == 0).
  C: O65 += V65_tile^T @ E bf16 (PSUM accumulate over k-tiles); row 64 =
     softmax denominator.
  D: y_tile = (O65_slice^T @ [Wo^T; bo]) * (1/den) in f32r -- den row makes
     the matmul add den*bo, so the per-partition reciprocal multiply yields
     O@Wo^T/den + bo exactly; y stored bf16 (host upcasts to f32).

Scheduling: A-stage chunk matmuls for the NEXT two ntiles and D-stage
output projections (512-col halves, alternating between the pkv and pq
PSUM banks) are interleaved between B/C pairs inside each slot so the
in-order PE stream never starves while ACT catches up on exp (ACT
throughput per pair ~1.04us vs PE ~0.7us). C trails two pairs behind
B/exp. V transposes are emitted at slot boundaries (po pool shared with
the O65 accumulators). Slot 3 column-splits its O65 accumulation so row
blocks 12/13 project while the high half still accumulates. Only DVE/ACT
may touch PSUM (GPSIMD may not); D normalizations go mostly to DVE with
the tail spread DVE/ACT. PE p-state is warmed with junk fp8 matmuls over
memset regions during the initial x DMA wait.
"""

import math

import numpy as np

D = 1024
H = 64
N = 4096
B = 4
CH = 64  # query chunk size (rows)
NT = 8  # ntiles of 512 keys
NEG = -1e10
# per masked-tile r: length of the fully-dead leading q-column prefix (min
# over both core parities). B widens each pair's two windows to the pair
# min so one merged exp per pair reads only matmul-covered PSUM.
TRUE_OFFS = [0, 128, 0, 128, 256, 384, 256, 384]
# per masked-tile r: end of the nonzero mask band (max over parities); the
# mask-add matmul only needs to cover [TRUE_OFFS[r], MASK_ENDS[r])
MASK_ENDS = [127, 255, 128, 256, 383, 511, 384, 512]

_PROG = None  # cached compiled program


# ---------------------------------------------------------------- host layout


def _key_order(s: int) -> np.ndarray:
    order = []
    for n in range(NT):
        mine = [8 * n + t for t in range(8) if t % 2 == s]
        theirs = [8 * n + t for t in range(8) if t % 2 != s]
        for c in mine + theirs:
            order.extend(range(CH * c, CH * c + CH))
    return np.array(order)


def _masks(s: int) -> np.ndarray:
    """fp8 mask tiles: [8, 128, 1024]; cols 0:512 hold 0/-240 (paired with a
    4*I fp8 identity -> -960 additive), cols 512:1024 are the zeroed second
    DoubleRow contraction tile."""
    ko = _key_order(s)
    qo = np.array([CH * c + i for c in range(s, 64, 2) for i in range(CH)])
    m = np.zeros((8, 128, 1024), dtype=np.float32)
    for r in range(8):
        keys = ko[128 * r : 128 * (r + 1)]
        qs = qo[0:512]
        m[r, :, 0:512] = np.where(keys[:, None] <= qs[None, :], 0.0, -240.0)
    return m


# ---------------------------------------------------------------- bass program


def _build():
    import concourse.mybir as mybir
    import concourse.tile as tile
    from concourse import bacc

    f32 = mybir.dt.float32
    f32r = mybir.dt.float32r
    bf16 = mybir.dt.bfloat16
    fp8 = mybir.dt.float8e4

    nc = bacc.Bacc("TRN2", target_bir_lowering=False, debug=False, num_devices=8)

    xh = nc.dram_tensor("xh", [NT, 128, 8, 512], bf16, kind="ExternalInput").ap()
    wkv = nc.dram_tensor("wkv", [9, 128, 128], bf16, kind="ExternalInput").ap()
    wq = nc.dram_tensor("wq", [9, 128, 64], bf16, kind="ExternalInput").ap()
    wobo = nc.dram_tensor("wobo", [65, 1024], f32r, kind="ExternalInput").ap()
    # [:, :, 0:512] = 0 / -240 causal mask, [:, :, 512:1024] = 0 (the dead
    # second DoubleRow tile); paired with a 4*I fp8 identity -> -960 additive
    masks = nc.dram_tensor("masks", [8, 128, 1024], fp8, kind="ExternalInput").ap()
    identb = nc.dram_tensor("identb", [128, 128], bf16, kind="ExternalInput").ap()
    id8 = nc.dram_tensor("id8", [128, 256], fp8, kind="ExternalInput").ap()
    ident = nc.dram_tensor("ident", [128, 128], f32r, kind="ExternalInput").ap()
    biases = nc.dram_tensor("biases", [128, 2], f32, kind="ExternalInput").ap()
    vones = nc.dram_tensor("vones", [128, 32, 1], bf16, kind="ExternalInput").ap()
    y = nc.dram_tensor("y", [2048, 1024], bf16, kind="ExternalOutput").ap()

    Exp = mybir.ActivationFunctionType.Exp
    Copy = mybir.ActivationFunctionType.Copy
    DR = mybir.MatmulPerfMode.DoubleRow
    mult = mybir.AluOpType.mult
    add_op = mybir.AluOpType.add
    scale = 1.0 / math.sqrt(D)

    with tile.TileContext(nc) as tc:
        with (
            tc.tile_pool(name="consts", bufs=1) as consts,
            tc.tile_pool(name="xp", bufs=4) as xpool,
            tc.tile_pool(name="ep", bufs=8) as epool,
            tc.tile_pool(name="yp", bufs=4) as ypool,
            tc.tile_pool(name="pkv", bufs=1, space="PSUM") as pkv,
            tc.tile_pool(name="pq", bufs=1, space="PSUM") as pq,
            tc.tile_pool(name="po", bufs=2, space="PSUM") as po,
            tc.tile_pool(name="ps", bufs=2, space="PSUM") as ps,
        ):
            # ---- persistent SBUF
            wkv_sb = consts.tile([128, 9 * 128], bf16)
            wq_sb = consts.tile([128, 9 * 64], bf16)
            id_sb = consts.tile([128, 128], f32r)
            idb_sb = consts.tile([128, 128], bf16)
            bias_sb = consts.tile([128, 2], f32)  # col 0: [bk|bov], col 1: bq
            vt_sb = consts.tile([128, N], bf16)  # rows 0:64 KT(bf16), 64:128 VT
            k8_sb = consts.tile([64, N + 128], fp8)  # KT fp8 + finite pad
            q8_sb = consts.tile([64, 2 * 2048], fp8)  # QT fp8 | zeros
            v65_sb = consts.tile([128, 32 * 65], bf16)
            id8_sb = consts.tile([128, 256], fp8)  # [4I | 0]
            mask_sb = consts.tile([128, 8 * 1024], fp8)
            wobo_sb = consts.tile([65, 1024], f32r)
            ot_sb = consts.tile([65, 2048], f32r)
            recip_sb = consts.tile([128, 16], f32)
            scratch_sb = consts.tile([1, 8], f32)

            # ---- constants, ordered by first use. Only what stage A needs
            # goes ahead of the first k8 casts in the in-order Pool queue;
            # the mask load rides ACT's DGE (ACT is idle until the first exp).
            wkv_v = wkv_sb[:].rearrange("p (c f) -> p c f", c=9)
            wkv_h = wkv.rearrange("c p f -> p c f")
            nc.sync.dma_start(wkv_v[:, 0:2], wkv_h[:, 0:2])
            nc.gpsimd.dma_start(wkv_v[:, 2:9], wkv_h[:, 2:9])
            nc.gpsimd.dma_start(
                wq_sb[:].rearrange("p (c f) -> p c f", c=9),
                wq.rearrange("c p f -> p c f"),
            )
            nc.gpsimd.dma_start(bias_sb[:], biases[:])

            # DoubleRow zero halves (DVE is idle at start); k8 pad first so
            # the ACT exp-table prewarm has initialized bytes to read
            nc.vector.memset(k8_sb[:, N : N + 128], 0.0)
            nc.vector.memset(q8_sb[:, 2048:4096], 0.0)

            # prewarm the ACT exp table while DMAs stream
            nc.scalar.activation(
                scratch_sb[:], k8_sb[0:1, N : N + 32].bitcast(f32), Exp, bias=0.0,
                scale=1.0,
            )
            nc.scalar.dma_start(
                mask_sb[:].rearrange("p (m f) -> p m f", m=8),
                masks.rearrange("m p f -> p m f"),
            )
            nc.scalar.dma_start(id8_sb[:], id8[:])

            def emit_late_consts():  # called after the first ntile's casts
                nc.gpsimd.dma_start(idb_sb[:], identb[:])
                nc.gpsimd.dma_start(
                    v65_sb[:].rearrange("p (t c) -> p t c", c=65)[:, :, 64:65],
                    vones[:],
                )
                nc.gpsimd.dma_start(wobo_sb[:], wobo[:])
                nc.gpsimd.dma_start(id_sb[:], ident[:])

            k8v = k8_sb[:].rearrange("p (t f) -> p t f", f=128)  # [64, 33, 128]
            q8v = q8_sb[:].rearrange("p (z f) -> p z f", z=2)  # [64, 2, 2048]

            # ---- A stage, chunk-granular so it can interleave into B/C slots
            a_state = {}

            def emit_a_dma(n, split=None):
                xn = xpool.tile([128, 4096], bf16, tag="x")
                xnv = xn[:].rearrange("p (c f) -> p c f", c=8)
                if split:
                    lo = 0
                    for hi in split:
                        nc.sync.dma_start(xnv[:, lo:hi], xh[n, :, lo:hi])
                        lo = hi
                else:
                    nc.sync.dma_start(xnv[:], xh[n])
                a_state[n] = xn

            def emit_a_chunk(n, c, kv_only=False, q_only=False):
                xn = a_state[n]
                if c == 0 and not q_only:
                    a_state[(n, "kv")] = pkv.tile(
                        [128, 512], f32, tag="kv", name="kvp"
                    )
                    a_state[(n, "q")] = pq.tile([64, 256], f32, tag="q", name="qp")
                kvp = a_state[(n, "kv")]
                qp = a_state[(n, "q")]
                if not q_only:
                    nc.tensor.matmul(
                        kvp[:],
                        wkv_sb[:, 128 * c : 128 * (c + 1)],
                        xn[:, 512 * c : 512 * c + 512],
                        start=(c == 0),
                        stop=(c == 7),
                    )
                if not kv_only:
                    nc.tensor.matmul(
                        qp[:],
                        wq_sb[:, 64 * c : 64 * (c + 1)],
                        xn[:, 512 * c : 512 * c + 256],
                        start=(c == 0),
                        stop=(c == 7),
                    )
                if kv_only and c == 7:
                    nc.vector.tensor_scalar(
                        out=vt_sb[:, 512 * n : 512 * (n + 1)],
                        in0=kvp[:],
                        scalar1=bias_sb[:, 0:1],
                        scalar2=None,
                        op0=add_op,
                    )
                    nc.gpsimd.tensor_copy(
                        k8_sb[:, 512 * n : 512 * (n + 1)],
                        vt_sb[0:64, 512 * n : 512 * (n + 1)],
                    )
                    return
                if q_only and c == 7:
                    nc.vector.tensor_scalar(
                        out=q8v[:, 0, 256 * n : 256 * (n + 1)],
                        in0=qp[:],
                        scalar1=bias_sb[0:64, 1:2],
                        scalar2=None,
                        op0=add_op,
                    )
                    return
                if c == 7 and not kv_only and not q_only:
                    nc.vector.tensor_scalar(
                        out=vt_sb[:, 512 * n : 512 * (n + 1)],
                        in0=kvp[:],
                        scalar1=bias_sb[:, 0:1],
                        scalar2=None,
                        op0=add_op,
                    )
                    nc.vector.tensor_scalar(
                        out=q8v[:, 0, 256 * n : 256 * (n + 1)],
                        in0=qp[:],
                        scalar1=bias_sb[0:64, 1:2],
                        scalar2=None,
                        op0=add_op,
                    )
                    # K bf16 -> fp8 recast on Pool (keeps DVE lean)
                    nc.gpsimd.tensor_copy(
                        k8_sb[:, 512 * n : 512 * (n + 1)],
                        vt_sb[0:64, 512 * n : 512 * (n + 1)],
                    )

            def emit_v_tiles(n):
                # V transposes for ntile n (po pool; only at slot boundaries,
                # when no O65 accumulator is live in the pool rotation)
                for t in range(4 * n, 4 * n + 4):
                    vp = po.tile([128, 64], bf16, tag="o")
                    nc.tensor.transpose(
                        vp[:],
                        vt_sb[64:128, 128 * t : 128 * (t + 1)],
                        idb_sb[64:128, 64:128],
                    )
                    nc.vector.tensor_copy(v65_sb[:, 65 * t : 65 * t + 64], vp[:])

            # ---- D stage: output projection, one 512-col half at a time so
            # it fills PSUM banks between B/C pairs. Alternates between the
            # pkv and pq banks (both projection-free by slot 3) so the PE
            # stream never waits on the previous half's normalization, and
            # spreads the PSUM->SBUF normalize over DVE/Pool/ACT.
            dh_n = [0]

            def emit_d_half(i, d):
                if d == 0:
                    a_state[("ys", i)] = ypool.tile(
                        [128, 1024], bf16, tag="y", name="ys"
                    )
                ys = a_state[("ys", i)]
                k = dh_n[0]
                dh_n[0] += 1
                if k % 2 == 0:
                    yp = pkv.tile([128, 512], f32, tag="kv", name="kvp")
                else:
                    yp = pq.tile([128, 512], f32, tag="q", name="qp")
                nc.tensor.matmul(
                    yp[:],
                    ot_sb[:, 128 * i : 128 * (i + 1)],
                    wobo_sb[:, 512 * d : 512 * (d + 1)],
                    start=True,
                    stop=True,
                )
                # NOTE: only DVE and ACT can read PSUM (GPSIMD cannot)
                engs = a_state.get("dh_engs", ("dve",))
                eng = engs[k % len(engs)]
                if eng == "dve":
                    nc.vector.tensor_scalar(
                        out=ys[:, 512 * d : 512 * (d + 1)],
                        in0=yp[:],
                        scalar1=recip_sb[:, i : i + 1],
                        scalar2=None,
                        op0=mult,
                    )
                else:
                    nc.scalar.activation(
                        ys[:, 512 * d : 512 * (d + 1)],
                        yp[:],
                        Copy,
                        bias=0.0,
                        scale=recip_sb[:, i : i + 1],
                    )
                if d == 1:
                    nc.gpsimd.dma_start(y[128 * i : 128 * (i + 1), :], ys[:])

            # denominator reciprocals (PE transposes of the O65 ones-row +
            # DVE reciprocal); emitted as a filler inside the next slot so
            # the PE transposes never stall on the slot-end ot copy
            def emit_recip(i0, ni):
                rp = pq.tile([128, 4], f32, tag="q", name="qp")
                for ii in range(ni):
                    i = i0 + ii
                    nc.tensor.transpose(
                        rp[:, ii : ii + 1],
                        ot_sb[64:65, 128 * i : 128 * (i + 1)].bitcast(f32),
                        id_sb[64:65, 64:65].bitcast(f32),
                    )
                nc.vector.reciprocal(recip_sb[:, i0 : i0 + ni], rp[:, 0:ni])

            def emit_bc(j):
                nk = 8 * (j + 1)
                ops = {}

                def get_op(key, shape):
                    # lazy allocation: the first C is two pairs in, so the
                    # V-transpose fillers ahead of it can rotate through the
                    # po pool without deadlocking against a live accumulator
                    if key not in ops:
                        ops[key] = po.tile([65, shape], f32, tag="o", name="op_")
                    return ops[key]

                # exp-independent PE filler work, emitted right before each C
                # so the in-order PE stream never waits on ACT
                fillers = []
                if j > 0:
                    fillers.append(("r", j - 1))
                if j < 3:
                    sp_ = (2, 4, 6, 8)
                    emit_a_dma(2 * j + 2, split=sp_)
                    emit_a_dma(2 * j + 3, split=sp_)
                    fillers.extend(("a", 2 * j + 2, c) for c in range(8))
                    if j == 2:
                        # row blocks 0/1 project in the pkv/pq window between
                        # the filler ntiles (keeps slot-3 DVE under the exp
                        # floor)
                        fillers.extend(
                            [("dh", 0, 0), ("dh", 1, 0), ("dh", 0, 1),
                             ("dh", 1, 1)]
                        )
                    fillers.extend(("a", 2 * j + 3, c) for c in range(8))
                else:
                    # slot 3: pkv/pq are free of projection work; stream the
                    # remaining row-blocks' output projections as fillers
                    for i in range(2, 12, 2):
                        fillers.extend(
                            [("dh", i, 0), ("dh", i + 1, 0),
                             ("dh", i, 1), ("dh", i + 1, 1)]
                        )

                npairs = nk // 2
                state = {"done": 0}

                def emit_fillers(p):
                    if j == 0:
                        # back-load: the filler A-ntile DMAs are still in
                        # flight early in short slots
                        want = len(fillers) * (p + 1) * (p + 1) // (npairs * npairs)
                    else:
                        want = len(fillers) * (p + 1) // npairs
                    while state["done"] < want:
                        f = fillers[state["done"]]
                        if f[0] == "a":
                            emit_a_chunk(f[1], f[2])
                        elif f[0] == "dh":
                            emit_d_half(f[1], f[2])
                        elif f[0] == "v":
                            emit_v_tiles(f[1])
                        else:
                            emit_recip(4 * f[1], 4)
                        state["done"] += 1

                def emit_c(t0, et, offs):
                    for h in range(2):
                        t = t0 + h
                        off = offs[h]
                        if j < 3:
                            nc.tensor.matmul(
                                get_op("op", 512)[:, off:512],
                                v65_sb[:, 65 * t : 65 * (t + 1)],
                                et[:, 512 * h + off : 512 * (h + 1)],
                                start=(t == 0),
                                stop=(t == nk - 1),
                            )
                            continue
                        if off < 256:
                            nc.tensor.matmul(
                                get_op("a", 256)[:, off:256],
                                v65_sb[:, 65 * t : 65 * (t + 1)],
                                et[:, 512 * h + off : 512 * h + 256],
                                start=(t == 0),
                                stop=(t == 27),
                            )
                        ob = max(off, 256) - 256
                        nc.tensor.matmul(
                            get_op("b", 256)[:, ob:256],
                            v65_sb[:, 65 * t : 65 * (t + 1)],
                            et[:, 512 * h + 256 + ob : 512 * (h + 1)],
                            start=(t == 0),
                            stop=(t == 31),
                        )

                # software-pipelined: C trails two pairs behind B/exp
                prevs = []
                for p in range(npairs):
                    t0 = 2 * p
                    sp = ps.tile([128, 1024], f32, tag="s")
                    offs = []
                    woff = 0  # pair-min B window so one merged exp is safe
                    rr0 = t0 - 8 * j
                    if rr0 >= 0:
                        woff = min(TRUE_OFFS[rr0], TRUE_OFFS[rr0 + 1])
                    for h in range(2):
                        t = t0 + h
                        rr = t - 8 * j
                        offs.append(0 if rr < 0 else TRUE_OFFS[rr])
                        nc.tensor.matmul(
                            sp[:, 512 * h + woff : 512 * (h + 1)],
                            k8v[:, t : t + 2, :],
                            q8v[:, :, 512 * j + woff : 512 * (j + 1)],
                            start=True,
                            stop=(rr < 0),
                            perf_mode=DR,
                        )
                        if rr >= 0:
                            off, end = TRUE_OFFS[rr], MASK_ENDS[rr]
                            mv = mask_sb[:].rearrange(
                                "p (m z f) -> p m z f", m=8, z=2
                            )
                            nc.tensor.matmul(
                                sp[:, 512 * h + off : 512 * h + end],
                                id8_sb[:].rearrange("p (z f) -> p z f", z=2),
                                mv[:, rr, :, off:end],
                                start=False,
                                stop=True,
                                perf_mode=DR,
                            )
                    et = epool.tile([128, 1024], bf16, tag="e")
                    if woff == 0:
                        nc.scalar.activation(et[:], sp[:], Exp, bias=0.0, scale=scale)
                    else:
                        nc.scalar.activation(
                            et[:].rearrange("p (b f) -> p b f", b=2)[:, :, woff:512],
                            sp[:].rearrange("p (b f) -> p b f", b=2)[:, :, woff:512],
                            Exp,
                            bias=0.0,
                            scale=scale,
                        )
                    prevs.append((t0, et, offs))
                    emit_fillers(p)
                    if len(prevs) > 2:
                        emit_c(*prevs.pop(0))
                if j < 3:
                    for pr in prevs:
                        emit_c(*pr)
                    nc.vector.tensor_copy(
                        ot_sb[:, 512 * j : 512 * (j + 1)], ops["op"][:]
                    )
                    return
                # slot-3 tail: low-half chain (ot copy, reciprocals, D12/13)
                # overlaps the high half's last C accumulations and exps
                emit_c(*prevs.pop(0))  # pair (28,29): b-half only
                nc.vector.tensor_copy(ot_sb[:, 1536:1792], ops["a"][:])
                emit_recip(12, 2)
                a_state["dh_engs"] = ("dve", "act", "dve", "dve")
                emit_d_half(12, 0)
                emit_d_half(12, 1)
                emit_c(*prevs.pop(0))  # pair (30,31): b-half only
                emit_d_half(13, 0)
                emit_d_half(13, 1)
                # single [65,256] copy feeds both the reciprocal chain (row
                # 64) and the D stationary reads
                nc.vector.tensor_copy(ot_sb[:, 1792:2048], ops["b"][:])
                emit_recip(14, 2)
                # last two row blocks: D14 wide on a free ps bank pair with
                # an ACT normalize; D15 split over pkv+pq with parallel DVE
                # normalizes. Tail y DMAs ride the idle SP queue.
                ys14 = ypool.tile([128, 1024], bf16, tag="y", name="ys")
                yp14 = ps.tile([128, 1024], f32, tag="s", name="sp")
                for d in range(2):
                    nc.tensor.matmul(
                        yp14[:, 512 * d : 512 * (d + 1)],
                        ot_sb[:, 128 * 14 : 128 * 15],
                        wobo_sb[:, 512 * d : 512 * (d + 1)],
                        start=True,
                        stop=True,
                    )
                nc.scalar.activation(
                    ys14[:], yp14[:], Copy, bias=0.0, scale=recip_sb[:, 14:15]
                )
                nc.sync.dma_start(y[128 * 14 : 128 * 15, :], ys14[:])
                ys15 = ypool.tile([128, 1024], bf16, tag="y", name="ys")
                for d in range(2):
                    if d == 0:
                        yp = pkv.tile([128, 512], f32, tag="kv", name="kvp")
                    else:
                        yp = pq.tile([128, 512], f32, tag="q", name="qp")
                    nc.tensor.matmul(
                        yp[:],
                        ot_sb[:, 128 * 15 : 128 * 16],
                        wobo_sb[:, 512 * d : 512 * (d + 1)],
                        start=True,
                        stop=True,
                    )
                    nc.vector.tensor_scalar(
                        out=ys15[:, 512 * d : 512 * (d + 1)], in0=yp[:],
                        scalar1=recip_sb[:, 15:16], scalar2=None, op0=mult,
                    )
                    if d == 0:
                        nc.sync.dma_start(y[128 * 15 : 128 * 16, 0:512],
                                          ys15[:, 0:512])
                    else:
                        nc.gpsimd.dma_start(y[128 * 15 : 128 * 16, 512:1024],
                                            ys15[:, 512:1024])

            # ---- emission. Ntile 0 runs all its KV matmuls before the Q
            # ones: KV needs only the first wkv chunks (tiny SP DMA) while Q
            # waits on the full wq load.
            emit_a_dma(0, split=(1, 2, 4, 8))
            emit_a_dma(1, split=(2, 4, 8))
            for c in range(8):
                emit_a_chunk(0, c, kv_only=True)
            for c in range(8):
                emit_a_chunk(0, c, q_only=True)
            emit_late_consts()
            for c in range(8):
                emit_a_chunk(1, c, kv_only=True)
            for c in range(8):
                emit_a_chunk(1, c, q_only=True)
            emit_v_tiles(0)
            emit_v_tiles(1)
            for j in range(4):
                emit_bc(j)
                if j < 3:
                    emit_v_tiles(2 * j + 2)
                    emit_v_tiles(2 * j + 3)

    nc.compile()
    return nc


def _get_prog():
    global _PROG
    if _PROG is None:
        _PROG = _build()
    return _PROG


# ---------------------------------------------------------------- entry point


def _xh(xb, korder):
    """[ntile, partition, chunk, 512] bf16 layout of x[b][korder].T."""
    import ml_dtypes

    xt = xb[korder].T  # [1024, 4096]
    return np.ascontiguousarray(
        xt.reshape(8, 128, 8, 512).transpose(2, 1, 0, 3).astype(ml_dtypes.bfloat16)
    )


def kernel(x, Wq, bq, Wk, bk, Wov, bov, Wo, bo, _trace=False):
    from concourse import bass_utils

    x = np.ascontiguousarray(np.asarray(x, dtype=np.float32))
    Wq = np.asarray(Wq, dtype=np.float32)
    bq = np.asarray(bq, dtype=np.float32)
    Wk = np.asarray(Wk, dtype=np.float32)
    bk = np.asarray(bk, dtype=np.float32)
    Wov = np.asarray(Wov, dtype=np.float32)
    bov = np.asarray(bov, dtype=np.float32)
    Wo = np.asarray(Wo, dtype=np.float32)
    bo = np.asarray(bo, dtype=np.float32)

    nc = _get_prog()

    wkv_arr = np.zeros((9, 128, 128), dtype=np.float32)
    wkv_t = np.concatenate([Wk, Wov], axis=0).T  # [1024, 128]
    for c in range(8):
        wkv_arr[c] = wkv_t[128 * c : 128 * (c + 1)]
    wkv_arr[8][0] = np.concatenate([bk, bov])

    wq_arr = np.zeros((9, 128, 64), dtype=np.float32)
    wq_t = Wq.T  # [1024, 64]
    for c in range(8):
        wq_arr[c] = wq_t[128 * c : 128 * (c + 1)]
    wq_arr[8][0] = bq

    import ml_dtypes

    wobo_arr = np.concatenate([Wo.T, bo[None, :]], axis=0)  # [65, 1024]
    wkv_arr = wkv_arr.astype(ml_dtypes.bfloat16)
    wq_arr = wq_arr.astype(ml_dtypes.bfloat16)
    biases_arr = np.zeros((128, 2), dtype=np.float32)
    biases_arr[:, 0] = np.concatenate([bk, bov])
    biases_arr[0:64, 1] = bq
    ident_arr = np.eye(128, dtype=np.float32)
    masks_s = [_masks(0), _masks(1)]
    korder_s = [_key_order(0), _key_order(1)]

    id8_arr = np.zeros((128, 256), dtype=np.float32)
    id8_arr[:, 0:128] = 4.0 * ident_arr

    in_maps = []
    for core in range(8):
        b, s = divmod(core, 2)
        in_maps.append(
            {
                "xh": _xh(x[b], korder_s[s]),
                "wkv": wkv_arr,
                "wq": wq_arr,
                "wobo": wobo_arr,
                "masks": masks_s[s].astype(ml_dtypes.float8_e4m3),
                "identb": ident_arr.astype(ml_dtypes.bfloat16),
                "id8": id8_arr.astype(ml_dtypes.float8_e4m3),
                "ident": ident_arr,
                "biases": biases_arr,
                "vones": np.ones((128, 32, 1), dtype=ml_dtypes.bfloat16),
            }
        )

    res = bass_utils.run_bass_kernel_spmd(
        nc, in_maps, core_ids=list(range(8)), trace=_trace
    )

    y = np.empty((B, N, D), dtype=np.float32)
    for core in range(8):
        b, s = divmod(core, 2)
        yc = np.asarray(res.results[core]["y"]).astype(np.float32)
        y[b].reshape(64, CH, D)[s::2] = yc.reshape(32, CH, D)
    return y


# revision 69
# speedup vs baseline: 1.0234x; 1.0234x over previous
"""Trainium2 Bass kernel for nn_AttentionHead (B=4, n_ctx=4096, d_model=1024,
d_hidden=64, causal, scale=1/sqrt(d_model)).

Sharding: 8 cores = 4 batches x 2 balanced causal shards. Core (b, s) handles
the 2048 query rows in 64-row chunks with chunk%2 == s. Keys/x-columns are
permuted per core (my-parity chunks first within each 512-key ntile) so that
every core runs the IDENTICAL SPMD program:

  - slot j (0..3) = 512 queries = my chunks of ntiles 2j, 2j+1
  - slot j attends k-tiles t = 0..8(j+1)-1 (128 permuted keys each)
  - k-tiles t < 8j are fully open; t = 8j + r (r in 0..7) get an additive
    causal mask that depends only on (r, s) -> 8 mask tiles per core, sent
    as data.

Per-core pipeline:
  A: KT/VT = [Wk;Wov] @ xT (bf16, weights stationary, PSUM-accumulated over
     8 d_model chunks); bias-add -> vt_sb bf16; K rows recast to fp8 (Pool)
     into k8_sb; Q likewise from each ntile's first 256 columns -> q8_sb fp8
     (with a zeroed second half for the DoubleRow trick). V transposed to
     natural [k,64] bf16 layout via PE transpose, with an appended ones
     column so attn@[V,1] also yields the softmax denominator.
  B: S^T[k,q] = K8_tile^T @ Q8_slot in ONE fp8 DoubleRow matmul per k-tile
     (second contraction tile = next K tile x zeros => 0.5 cycles/row, 2x
     over bf16); additive -960 mask (fp8 -240 tiles against a 4*I fp8
     identity, DoubleRow again) for diagonal tiles; exp((S+M)/32) on ACT
     over the pair -> SBUF bf16 (no row-max subtraction needed:
     |scores/32| <~ 1.5, and exp(-960/32) == 0).
  C: O65 += V65_tile^T @ E bf16 (PSUM accumulate over k-tiles); row 64 =
     softmax denominator.
  D: y_tile = (O65_slice^T @ [Wo^T; bo]) * (1/den) in f32r -- den row makes
     the matmul add den*bo, so the per-partition reciprocal multiply yields
     O@Wo^T/den + bo exactly; y stored bf16 (host upcasts to f32).

Scheduling: A-stage chunk matmuls for the NEXT two ntiles and D-stage
output projections (512-col halves, alternating between the pkv and pq
PSUM banks) are interleaved between B/C pairs inside each slot so the
in-order PE stream never starves while ACT catches up on exp (ACT
throughput per pair ~1.04us vs PE ~0.7us). C trails two pairs behind
B/exp. V transposes are emitted at slot boundaries (po pool shared with
the O65 accumulators). Slot 3 column-splits its O65 accumulation so row
blocks 12/13 project while the high half still accumulates. Only DVE/ACT
may touch PSUM (GPSIMD may not); D normalizations go mostly to DVE with
the tail spread DVE/ACT. PE p-state is warmed with junk fp8 matmuls over
memset regions during the initial x DMA wait.
"""

import math

import numpy as np

D = 1024
H = 64
N = 4096
B = 4
CH = 64  # query chunk size (rows)
NT = 8  # ntiles of 512 keys
NEG = -1e10
# per masked-tile r: length of the fully-dead leading q-column prefix (min
# over both core parities). B widens each pair's two windows to the pair
# min so one merged exp per pair reads only matmul-covered PSUM.
TRUE_OFFS = [0, 128, 0, 128, 256, 384, 256, 384]
# per masked-tile r: end of the nonzero mask band (max over parities); the
# mask-add matmul only needs to cover [TRUE_OFFS[r], MASK_ENDS[r])
MASK_ENDS = [127, 255, 128, 256, 383, 511, 384, 512]

_PROG = None  # cached compiled program


# ---------------------------------------------------------------- host layout


def _key_order(s: int) -> np.ndarray:
    order = []
    for n in range(NT):
        mine = [8 * n + t for t in range(8) if t % 2 == s]
        theirs = [8 * n + t for t in range(8) if t % 2 != s]
        for c in mine + theirs:
            order.extend(range(CH * c, CH * c + CH))
    return np.array(order)


def _masks(s: int) -> np.ndarray:
    """fp8 mask tiles: [8, 128, 1024]; cols 0:512 hold 0/-240 (paired with a
    4*I fp8 identity -> -960 additive), cols 512:1024 are the zeroed second
    DoubleRow contraction tile."""
    ko = _key_order(s)
    qo = np.array([CH * c + i for c in range(s, 64, 2) for i in range(CH)])
    m = np.zeros((8, 128, 1024), dtype=np.float32)
    for r in range(8):
        keys = ko[128 * r : 128 * (r + 1)]
        qs = qo[0:512]
        m[r, :, 0:512] = np.where(keys[:, None] <= qs[None, :], 0.0, -240.0)
    return m


# ---------------------------------------------------------------- bass program


def _build():
    import concourse.mybir as mybir
    import concourse.tile as tile
    from concourse import bacc

    f32 = mybir.dt.float32
    f32r = mybir.dt.float32r
    bf16 = mybir.dt.bfloat16
    fp8 = mybir.dt.float8e4

    nc = bacc.Bacc("TRN2", target_bir_lowering=False, debug=False, num_devices=8)

    xh = nc.dram_tensor("xh", [NT, 128, 8, 512], bf16, kind="ExternalInput").ap()
    wkv = nc.dram_tensor("wkv", [9, 128, 128], bf16, kind="ExternalInput").ap()
    wq = nc.dram_tensor("wq", [9, 128, 64], bf16, kind="ExternalInput").ap()
    wobo = nc.dram_tensor("wobo", [65, 1024], f32r, kind="ExternalInput").ap()
    # [:, :, 0:512] = 0 / -240 causal mask, [:, :, 512:1024] = 0 (the dead
    # second DoubleRow tile); paired with a 4*I fp8 identity -> -960 additive
    masks = nc.dram_tensor("masks", [8, 128, 1024], fp8, kind="ExternalInput").ap()
    identb = nc.dram_tensor("identb", [128, 128], bf16, kind="ExternalInput").ap()
    id8 = nc.dram_tensor("id8", [128, 256], fp8, kind="ExternalInput").ap()
    ident = nc.dram_tensor("ident", [128, 128], f32r, kind="ExternalInput").ap()
    biases = nc.dram_tensor("biases", [128, 2], f32, kind="ExternalInput").ap()
    vones = nc.dram_tensor("vones", [128, 32, 1], bf16, kind="ExternalInput").ap()
    y = nc.dram_tensor("y", [2048, 1024], bf16, kind="ExternalOutput").ap()

    Exp = mybir.ActivationFunctionType.Exp
    Copy = mybir.ActivationFunctionType.Copy
    DR = mybir.MatmulPerfMode.DoubleRow
    mult = mybir.AluOpType.mult
    add_op = mybir.AluOpType.add
    scale = 1.0 / math.sqrt(D)

    with tile.TileContext(nc) as tc:
        with (
            tc.tile_pool(name="consts", bufs=1) as consts,
            tc.tile_pool(name="xp", bufs=4) as xpool,
            tc.tile_pool(name="ep", bufs=8) as epool,
            tc.tile_pool(name="yp", bufs=4) as ypool,
            tc.tile_pool(name="pkv", bufs=1, space="PSUM") as pkv,
            tc.tile_pool(name="pq", bufs=1, space="PSUM") as pq,
            tc.tile_pool(name="po", bufs=2, space="PSUM") as po,
            tc.tile_pool(name="ps", bufs=2, space="PSUM") as ps,
        ):
            # ---- persistent SBUF
            wkv_sb = consts.tile([128, 9 * 128], bf16)
            wq_sb = consts.tile([128, 9 * 64], bf16)
            id_sb = consts.tile([128, 128], f32r)
            idb_sb = consts.tile([128, 128], bf16)
            bias_sb = consts.tile([128, 2], f32)  # col 0: [bk|bov], col 1: bq
            vt_sb = consts.tile([128, N], bf16)  # rows 0:64 KT(bf16), 64:128 VT
            k8_sb = consts.tile([64, N + 128], fp8)  # KT fp8 + finite pad
            q8_sb = consts.tile([64, 2 * 2048], fp8)  # QT fp8 | zeros
            v65_sb = consts.tile([128, 32 * 65], bf16)
            id8_sb = consts.tile([128, 256], fp8)  # [4I | 0]
            mask_sb = consts.tile([128, 8 * 1024], fp8)
            wobo_sb = consts.tile([65, 1024], f32r)
            ot_sb = consts.tile([65, 2048], f32r)
            recip_sb = consts.tile([128, 16], f32)
            scratch_sb = consts.tile([1, 8], f32)

            # ---- constants, ordered by first use. Only what stage A needs
            # goes ahead of the first k8 casts in the in-order Pool queue;
            # the mask load rides ACT's DGE (ACT is idle until the first exp).
            wkv_v = wkv_sb[:].rearrange("p (c f) -> p c f", c=9)
            wkv_h = wkv.rearrange("c p f -> p c f")
            nc.sync.dma_start(wkv_v[:, 0:2], wkv_h[:, 0:2])
            nc.gpsimd.dma_start(wkv_v[:, 2:9], wkv_h[:, 2:9])
            nc.gpsimd.dma_start(
                wq_sb[:].rearrange("p (c f) -> p c f", c=9),
                wq.rearrange("c p f -> p c f"),
            )
            nc.gpsimd.dma_start(bias_sb[:], biases[:])

            # DoubleRow zero halves (DVE is idle at start); k8 pad first so
            # the ACT exp-table prewarm has initialized bytes to read
            nc.vector.memset(k8_sb[:, N : N + 128], 0.0)
            nc.vector.memset(q8_sb[:, 2048:4096], 0.0)

            # prewarm the ACT exp table while DMAs stream
            nc.scalar.activation(
                scratch_sb[:], k8_sb[0:1, N : N + 32].bitcast(f32), Exp, bias=0.0,
                scale=1.0,
            )
            nc.scalar.dma_start(
                mask_sb[:].rearrange("p (m f) -> p m f", m=8),
                masks.rearrange("m p f -> p m f"),
            )
            nc.scalar.dma_start(id8_sb[:], id8[:])

            def emit_late_consts():  # called after the first ntile's casts
                nc.gpsimd.dma_start(idb_sb[:], identb[:])
                nc.gpsimd.dma_start(
                    v65_sb[:].rearrange("p (t c) -> p t c", c=65)[:, :, 64:65],
                    vones[:],
                )
                nc.gpsimd.dma_start(wobo_sb[:], wobo[:])
                nc.gpsimd.dma_start(id_sb[:], ident[:])

            k8v = k8_sb[:].rearrange("p (t f) -> p t f", f=128)  # [64, 33, 128]
            q8v = q8_sb[:].rearrange("p (z f) -> p z f", z=2)  # [64, 2, 2048]

            # ---- A stage, chunk-granular so it can interleave into B/C slots
            a_state = {}

            def emit_a_dma(n, split=None):
                xn = xpool.tile([128, 4096], bf16, tag="x")
                xnv = xn[:].rearrange("p (c f) -> p c f", c=8)
                if split:
                    lo = 0
                    for hi in split:
                        nc.sync.dma_start(xnv[:, lo:hi], xh[n, :, lo:hi])
                        lo = hi
                else:
                    nc.sync.dma_start(xnv[:], xh[n])
                a_state[n] = xn

            def emit_a_chunk(n, c, kv_only=False, q_only=False):
                xn = a_state[n]
                if c == 0 and not q_only:
                    a_state[(n, "kv")] = pkv.tile(
                        [128, 512], f32, tag="kv", name="kvp"
                    )
                    a_state[(n, "q")] = pq.tile([64, 256], f32, tag="q", name="qp")
                kvp = a_state[(n, "kv")]
                qp = a_state[(n, "q")]
                if not q_only:
                    nc.tensor.matmul(
                        kvp[:],
                        wkv_sb[:, 128 * c : 128 * (c + 1)],
                        xn[:, 512 * c : 512 * c + 512],
                        start=(c == 0),
                        stop=(c == 7),
                    )
                if not kv_only:
                    nc.tensor.matmul(
                        qp[:],
                        wq_sb[:, 64 * c : 64 * (c + 1)],
                        xn[:, 512 * c : 512 * c + 256],
                        start=(c == 0),
                        stop=(c == 7),
                    )
                if kv_only and c == 7:
                    nc.vector.tensor_scalar(
                        out=vt_sb[:, 512 * n : 512 * (n + 1)],
                        in0=kvp[:],
                        scalar1=bias_sb[:, 0:1],
                        scalar2=None,
                        op0=add_op,
                    )
                    nc.gpsimd.tensor_copy(
                        k8_sb[:, 512 * n : 512 * (n + 1)],
                        vt_sb[0:64, 512 * n : 512 * (n + 1)],
                    )
                    return
                if q_only and c == 7:
                    nc.vector.tensor_scalar(
                        out=q8v[:, 0, 256 * n : 256 * (n + 1)],
                        in0=qp[:],
                        scalar1=bias_sb[0:64, 1:2],
                        scalar2=None,
                        op0=add_op,
                    )
                    return
                if c == 7 and not kv_only and not q_only:
                    nc.vector.tensor_scalar(
                        out=vt_sb[:, 512 * n : 512 * (n + 1)],
                        in0=kvp[:],
                        scalar1=bias_sb[:, 0:1],
                        scalar2=None,
                        op0=add_op,
                    )
                    nc.vector.tensor_scalar(
                        out=q8v[:, 0, 256 * n : 256 * (n + 1)],
                        in0=qp[:],
                        scalar1=bias_sb[0:64, 1:2],
                        scalar2=None,
                        op0=add_op,
                    )
                    # K bf16 -> fp8 recast on Pool (keeps DVE lean)
                    nc.gpsimd.tensor_copy(
                        k8_sb[:, 512 * n : 512 * (n + 1)],
                        vt_sb[0:64, 512 * n : 512 * (n + 1)],
                    )

            def emit_v_tiles(n):
                # V transposes for ntile n (po pool; only at slot boundaries,
                # when no O65 accumulator is live in the pool rotation)
                for t in range(4 * n, 4 * n + 4):
                    vp = po.tile([128, 64], bf16, tag="o")
                    nc.tensor.transpose(
                        vp[:],
                        vt_sb[64:128, 128 * t : 128 * (t + 1)],
                        idb_sb[64:128, 64:128],
                    )
                    nc.vector.tensor_copy(v65_sb[:, 65 * t : 65 * t + 64], vp[:])

            # ---- D stage: output projection, one 512-col half at a time so
            # it fills PSUM banks between B/C pairs. Alternates between the
            # pkv and pq banks (both projection-free by slot 3) so the PE
            # stream never waits on the previous half's normalization, and
            # spreads the PSUM->SBUF normalize over DVE/Pool/ACT.
            dh_n = [0]

            def emit_d_half(i, d):
                if d == 0:
                    a_state[("ys", i)] = ypool.tile(
                        [128, 1024], bf16, tag="y", name="ys"
                    )
                ys = a_state[("ys", i)]
                k = dh_n[0]
                dh_n[0] += 1
                if k % 2 == 0:
                    yp = pkv.tile([128, 512], f32, tag="kv", name="kvp")
                else:
                    yp = pq.tile([128, 512], f32, tag="q", name="qp")
                nc.tensor.matmul(
                    yp[:],
                    ot_sb[:, 128 * i : 128 * (i + 1)],
                    wobo_sb[:, 512 * d : 512 * (d + 1)],
                    start=True,
                    stop=True,
                )
                # NOTE: only DVE and ACT can read PSUM (GPSIMD cannot)
                engs = a_state.get("dh_engs", ("dve",))
                eng = engs[k % len(engs)]
                if eng == "dve":
                    nc.vector.tensor_scalar(
                        out=ys[:, 512 * d : 512 * (d + 1)],
                        in0=yp[:],
                        scalar1=recip_sb[:, i : i + 1],
                        scalar2=None,
                        op0=mult,
                    )
                else:
                    nc.scalar.activation(
                        ys[:, 512 * d : 512 * (d + 1)],
                        yp[:],
                        Copy,
                        bias=0.0,
                        scale=recip_sb[:, i : i + 1],
                    )
                if d == 1:
                    nc.gpsimd.dma_start(y[128 * i : 128 * (i + 1), :], ys[:])

            # denominator reciprocals (PE transposes of the O65 ones-row +
            # DVE reciprocal); emitted as a filler inside the next slot so
            # the PE transposes never stall on the slot-end ot copy
            def emit_recip(i0, ni):
                rp = pq.tile([128, 4], f32, tag="q", name="qp")
                for ii in range(ni):
                    i = i0 + ii
                    nc.tensor.transpose(
                        rp[:, ii : ii + 1],
                        ot_sb[64:65, 128 * i : 128 * (i + 1)].bitcast(f32),
                        id_sb[64:65, 64:65].bitcast(f32),
                    )
                nc.vector.reciprocal(recip_sb[:, i0 : i0 + ni], rp[:, 0:ni])

            def emit_bc(j):
                nk = 8 * (j + 1)
                ops = {}

                def get_op(key, shape):
                    # lazy allocation: the first C is two pairs in, so the
                    # V-transpose fillers ahead of it can rotate through the
                    # po pool without deadlocking against a live accumulator
                    if key not in ops:
                        ops[key] = po.tile([65, shape], f32, tag="o", name="op_")
                    return ops[key]

                # exp-independent PE filler work, emitted right before each C
                # so the in-order PE stream never waits on ACT
                fillers = []
                if j > 0:
                    fillers.append(("r", j - 1))
                if j < 3:
                    sp_ = (2, 4, 6, 8)
                    emit_a_dma(2 * j + 2, split=sp_)
                    emit_a_dma(2 * j + 3, split=sp_)
                    fillers.extend(("a", 2 * j + 2, c) for c in range(8))
                    if j == 2:
                        # row blocks 0/1 project in the pkv/pq window between
                        # the filler ntiles (keeps slot-3 DVE under the exp
                        # floor)
                        fillers.extend(
                            [("dh", 0, 0), ("dh", 1, 0), ("dh", 0, 1),
                             ("dh", 1, 1)]
                        )
                    fillers.extend(("a", 2 * j + 3, c) for c in range(8))
                else:
                    # slot 3: pkv/pq are free of projection work; stream the
                    # remaining row-blocks' output projections as fillers
                    for i in range(2, 12, 2):
                        fillers.extend(
                            [("dh", i, 0), ("dh", i + 1, 0),
                             ("dh", i, 1), ("dh", i + 1, 1)]
                        )

                npairs = nk // 2
                state = {"done": 0}

                def emit_fillers(p):
                    if j == 0:
                        # back-load: the filler A-ntile DMAs are still in
                        # flight early in short slots
                        want = len(fillers) * (p + 1) * (p + 1) // (npairs * npairs)
                    else:
                        want = len(fillers) * (p + 1) // npairs
                    while state["done"] < want:
                        f = fillers[state["done"]]
                        if f[0] == "a":
                            emit_a_chunk(f[1], f[2])
                        elif f[0] == "dh":
                            emit_d_half(f[1], f[2])
                        elif f[0] == "v":
                            emit_v_tiles(f[1])
                        else:
                            emit_recip(4 * f[1], 4)
                        state["done"] += 1

                def emit_c(t0, et, offs):
                    for h in range(2):
                        t = t0 + h
                        off = offs[h]
                        if j < 3:
                            nc.tensor.matmul(
                                get_op("op", 512)[:, off:512],
                                v65_sb[:, 65 * t : 65 * (t + 1)],
                                et[:, 512 * h + off : 512 * (h + 1)],
                                start=(t == 0),
                                stop=(t == nk - 1),
                            )
                            continue
                        if off < 256:
                            nc.tensor.matmul(
                                get_op("a", 256)[:, off:256],
                                v65_sb[:, 65 * t : 65 * (t + 1)],
                                et[:, 512 * h + off : 512 * h + 256],
                                start=(t == 0),
                                stop=(t == 27),
                            )
                        ob = max(off, 256) - 256
                        nc.tensor.matmul(
                            get_op("b", 256)[:, ob:256],
                            v65_sb[:, 65 * t : 65 * (t + 1)],
                            et[:, 512 * h + 256 + ob : 512 * (h + 1)],
                            start=(t == 0),
                            stop=(t == 31),
                        )

                # software-pipelined: C trails two pairs behind B/exp
                prevs = []
                for p in range(npairs):
                    t0 = 2 * p
                    sp = ps.tile([128, 1024], f32, tag="s")
                    offs = []
                    woff = 0  # pair-min B window so one merged exp is safe
                    rr0 = t0 - 8 * j
                    if rr0 >= 0:
                        woff = min(TRUE_OFFS[rr0], TRUE_OFFS[rr0 + 1])
                    for h in range(2):
                        t = t0 + h
                        rr = t - 8 * j
                        offs.append(0 if rr < 0 else TRUE_OFFS[rr])
                        nc.tensor.matmul(
                            sp[:, 512 * h + woff : 512 * (h + 1)],
                            k8v[:, t : t + 2, :],
                            q8v[:, :, 512 * j + woff : 512 * (j + 1)],
                            start=True,
                            stop=(rr < 0),
                            perf_mode=DR,
                        )
                        if rr >= 0:
                            off, end = TRUE_OFFS[rr], MASK_ENDS[rr]
                            mv = mask_sb[:].rearrange(
                                "p (m z f) -> p m z f", m=8, z=2
                            )
                            nc.tensor.matmul(
                                sp[:, 512 * h + off : 512 * h + end],
                                id8_sb[:].rearrange("p (z f) -> p z f", z=2),
                                mv[:, rr, :, off:end],
                                start=False,
                                stop=True,
                                perf_mode=DR,
                            )
                    et = epool.tile([128, 1024], bf16, tag="e")
                    if woff == 0:
                        nc.scalar.activation(et[:], sp[:], Exp, bias=0.0, scale=scale)
                    else:
                        nc.scalar.activation(
                            et[:].rearrange("p (b f) -> p b f", b=2)[:, :, woff:512],
                            sp[:].rearrange("p (b f) -> p b f", b=2)[:, :, woff:512],
                            Exp,
                            bias=0.0,
                            scale=scale,
                        )
                    prevs.append((t0, et, offs))
                    emit_fillers(p)
                    if len(prevs) > 2:
                        emit_c(*prevs.pop(0))
                if j < 3:
                    for pr in prevs:
                        emit_c(*pr)
                    nc.vector.tensor_copy(
                        ot_sb[:, 512 * j : 512 * (j + 1)], ops["op"][:]
                    )
                    return
                # slot-3 tail: low-half chain (ot copy, reciprocals, D12/13)
                # overlaps the high half's last C accumulations and exps
                emit_c(*prevs.pop(0))  # pair (28,29): b-half only
                nc.vector.tensor_copy(ot_sb[:, 1536:1792], ops["a"][:])
                emit_recip(12, 2)
                a_state["dh_engs"] = ("dve", "act", "dve", "dve")
                emit_d_half(12, 0)
                emit_d_half(12, 1)
                emit_c(*prevs.pop(0))  # pair (30,31): b-half only
                emit_d_half(13, 0)
                emit_d_half(13, 1)
                # single [65,256] copy feeds both the reciprocal chain (row
                # 64) and the D stationary reads
                nc.vector.tensor_copy(ot_sb[:, 1792:2048], ops["b"][:])
                emit_recip(14, 2)
                # last two row blocks: D14 wide on a free ps bank pair with
                # an ACT normalize; D15 split over pkv+pq with parallel DVE
                # normalizes. Tail y DMAs ride the idle SP queue.
                ys14 = ypool.tile([128, 1024], bf16, tag="y", name="ys")
                yp14 = ps.tile([128, 1024], f32, tag="s", name="sp")
                for d in range(2):
                    nc.tensor.matmul(
                        yp14[:, 512 * d : 512 * (d + 1)],
                        ot_sb[:, 128 * 14 : 128 * 15],
                        wobo_sb[:, 512 * d : 512 * (d + 1)],
                        start=True,
                        stop=True,
                    )
                nc.scalar.activation(
                    ys14[:], yp14[:], Copy, bias=0.0, scale=recip_sb[:, 14:15]
                )
                nc.sync.dma_start(y[128 * 14 : 128 * 15, :], ys14[:])
                ys15 = ypool.tile([128, 1024], bf16, tag="y", name="ys")
                for d in range(2):
                    if d == 0:
                        yp = pkv.tile([128, 512], f32, tag="kv", name="kvp")
                    else:
                        yp = pq.tile([128, 512], f32, tag="q", name="qp")
                    nc.tensor.matmul(
                        yp[:],
                        ot_sb[:, 128 * 15 : 128 * 16],
                        wobo_sb[:, 512 * d : 512 * (d + 1)],
                        start=True,
                        stop=True,
                    )
                    nc.vector.tensor_scalar(
                        out=ys15[:, 512 * d : 512 * (d + 1)], in0=yp[:],
                        scalar1=recip_sb[:, 15:16], scalar2=None, op0=mult,
                    )
                    if d == 0:
                        nc.sync.dma_start(y[128 * 15 : 128 * 16, 0:512],
                                          ys15[:, 0:512])
                    else:
                        nc.gpsimd.dma_start(y[128 * 15 : 128 * 16, 512:1024],
                                            ys15[:, 512:1024])

            # ---- emission. Ntile 0 runs all its KV matmuls before the Q
            # ones: KV needs only the first wkv chunks (tiny SP DMA) while Q
            # waits on the full wq load.
            emit_a_dma(0, split=(1, 2, 4, 8))
            # p-state warmup: one junk fp8 matmul over the memset zero
            # regions starts the PE clock ramp during the x0 DMA wait
            wp = ps.tile([128, 768], f32, tag="s", name="sp")
            nc.tensor.matmul(
                wp[:, 0:512],
                k8_sb[:, N : N + 128],
                q8_sb[:, 2048 : 2048 + 512],
                start=True,
                stop=True,
            )
            emit_a_dma(1, split=(2, 4, 8))
            for c in range(8):
                emit_a_chunk(0, c, kv_only=True)
            for c in range(8):
                emit_a_chunk(0, c, q_only=True)
            emit_late_consts()
            for c in range(8):
                emit_a_chunk(1, c, kv_only=True)
            for c in range(8):
                emit_a_chunk(1, c, q_only=True)
            emit_v_tiles(0)
            emit_v_tiles(1)
            for j in range(4):
                emit_bc(j)
                if j < 3:
                    emit_v_tiles(2 * j + 2)
                    emit_v_tiles(2 * j + 3)

    nc.compile()
    return nc


def _get_prog():
    global _PROG
    if _PROG is None:
        _PROG = _build()
    return _PROG


# ---------------------------------------------------------------- entry point


def _xh(xb, korder):
    """[ntile, partition, chunk, 512] bf16 layout of x[b][korder].T."""
    import ml_dtypes

    xt = xb[korder].T  # [1024, 4096]
    return np.ascontiguousarray(
        xt.reshape(8, 128, 8, 512).transpose(2, 1, 0, 3).astype(ml_dtypes.bfloat16)
    )


def kernel(x, Wq, bq, Wk, bk, Wov, bov, Wo, bo, _trace=False):
    from concourse import bass_utils

    x = np.ascontiguousarray(np.asarray(x, dtype=np.float32))
    Wq = np.asarray(Wq, dtype=np.float32)
    bq = np.asarray(bq, dtype=np.float32)
    Wk = np.asarray(Wk, dtype=np.float32)
    bk = np.asarray(bk, dtype=np.float32)
    Wov = np.asarray(Wov, dtype=np.float32)
    bov = np.asarray(bov, dtype=np.float32)
    Wo = np.asarray(Wo, dtype=np.float32)
    bo = np.asarray(bo, dtype=np.float32)

    nc = _get_prog()

    wkv_arr = np.zeros((9, 128, 128), dtype=np.float32)
    wkv_t = np.concatenate([Wk, Wov], axis=0).T  # [1024, 128]
    for c in range(8):
        wkv_arr[c] = wkv_t[128 * c : 128 * (c + 1)]
    wkv_arr[8][0] = np.concatenate([bk, bov])

    wq_arr = np.zeros((9, 128, 64), dtype=np.float32)
    wq_t = Wq.T  # [1024, 64]
    for c in range(8):
        wq_arr[c] = wq_t[128 * c : 128 * (c + 1)]
    wq_arr[8][0] = bq

    import ml_dtypes

    wobo_arr = np.concatenate([Wo.T, bo[None, :]], axis=0)  # [65, 1024]
    wkv_arr = wkv_arr.astype(ml_dtypes.bfloat16)
    wq_arr = wq_arr.astype(ml_dtypes.bfloat16)
    biases_arr = np.zeros((128, 2), dtype=np.float32)
    biases_arr[:, 0] = np.concatenate([bk, bov])
    biases_arr[0:64, 1] = bq
    ident_arr = np.eye(128, dtype=np.float32)
    masks_s = [_masks(0), _masks(1)]
    korder_s = [_key_order(0), _key_order(1)]

    id8_arr = np.zeros((128, 256), dtype=np.float32)
    id8_arr[:, 0:128] = 4.0 * ident_arr

    in_maps = []
    for core in range(8):
        b, s = divmod(core, 2)
        in_maps.append(
            {
                "xh": _xh(x[b], korder_s[s]),
                "wkv": wkv_arr,
                "wq": wq_arr,
                "wobo": wobo_arr,
                "masks": masks_s[s].astype(ml_dtypes.float8_e4m3),
                "identb": ident_arr.astype(ml_dtypes.bfloat16),
                "id8": id8_arr.astype(ml_dtypes.float8_e4m3),
                "ident": ident_arr,
                "biases": biases_arr,
                "vones": np.ones((128, 32, 1), dtype=ml_dtypes.bfloat16),
            }
        )

    res = bass_utils.run_bass_kernel_spmd(
        nc, in_maps, core_ids=list(range(8)), trace=_trace
    )

    y = np.empty((B, N, D), dtype=np.float32)
    for core in range(8):
        b, s = divmod(core, 2)
        yc = np.asarray(res.results[core]["y"]).astype(np.float32)
        y[b].reshape(64, CH, D)[s::2] = yc.reshape(32, CH, D)
    return y
